# revision 5
# baseline (speedup 1.0000x reference)
"""Multi-head attention (B=8, N=1024, D=1024, H=16, Dh=64) on 8 TRN2 NeuronCores.

Sharding: pure data-parallel over batch — core i computes batch element i
end-to-end; weights are replicated. No collectives.

Per-core dataflow (all matmuls in float32r, full PE rate):
  A: xT = x.T                       (PE transpose via identity)
  C: v  = x @ Wv                    (lhsT = xT tiles)   [n, h, dv] +ones col
  B: qkT[a, n] = (x @ Wqk).T        (lhsT = Wqk tiles, rhs = xT)
  D: per head h: scoresT[j, i] = kT_h.T @ qT_h ; attnT = exp(scoresT/8) (ACT)
     avT[dv, i] = sum_j [v_h | 1] attnT  -> row 64 = softmax denominators
     mergedT[a, n] = avT * (1/denominator)   (DVE, gpsimd partition_broadcast)
  E: out[n, e] = mergedT.T @ Wout + b_out
"""

import sys

sys.path.insert(0, "/opt/trn_rl_repo")

import numpy as np

B, N, DIM = 8, 1024, 1024
HEADS, DH = 16, 64
P = 128
T = N // P  # 8 tiles per 1024 dim
SCALE = DH**-0.5

_CACHE = {}


def _build():
    import concourse.tile as tile
    from concourse import bacc, mybir
    from concourse.masks import make_identity

    F32 = mybir.dt.float32
    F32R = mybir.dt.float32r
    Exp = mybir.ActivationFunctionType.Exp
    mult = mybir.AluOpType.mult
    add = mybir.AluOpType.add

    nc = bacc.Bacc("TRN2", target_bir_lowering=False, debug=False, num_devices=8)
    x_ext = nc.declare_dram_parameter("x", [N, DIM], F32, isOutput=False)
    wqkv_ext = nc.declare_dram_parameter("w_qkv", [DIM, 3 * DIM], F32, isOutput=False)
    wout_ext = nc.declare_dram_parameter("w_out", [DIM, DIM], F32, isOutput=False)
    bout_ext = nc.declare_dram_parameter("b_out", [1, DIM], F32, isOutput=False)
    out_ext = nc.declare_dram_parameter("out", [N, DIM], F32, isOutput=True)

    def r(ap):
        return ap.bitcast(F32R)

    with tile.TileContext(nc) as tc:
        with (
            tc.tile_pool(name="const", bufs=1) as const,
            tc.tile_pool(name="merged", bufs=1) as merged_pool,
        ):
            ident = const.tile([P, P], F32, tag="ident")
            make_identity(nc, ident)
            mergedT = [merged_pool.tile([P, N], F32R, tag=f"m{a}", name=f"m{a}") for a in range(T)]

            with (
                tc.tile_pool(name="xt", bufs=1) as xt_pool,
                tc.tile_pool(name="vp", bufs=1) as v_pool,
            ):
                xT = [xt_pool.tile([P, N], F32R, tag=f"xT{t}", name=f"xT{t}") for t in range(T)]
                v = [v_pool.tile([P, HEADS, DH + 1], F32R, tag=f"v{t}", name=f"v{t}") for t in range(T)]

                # ---- Phase A: transpose x -> xT ----
                with (
                    tc.tile_pool(name="xin", bufs=3) as xin,
                    tc.tile_pool(name="pst", bufs=4, space="PSUM") as pst,
                ):
                    for nt in range(T):
                        x_sb = xin.tile([P, DIM], F32, tag="x_sb")
                        nc.sync.dma_start(x_sb[:], x_ext[nt * P : (nt + 1) * P, :])
                        for dt in range(T):
                            ps = pst.tile([P, P], F32, tag="pst")
                            nc.tensor.transpose(
                                ps[:], x_sb[:, dt * P : (dt + 1) * P], ident[:]
                            )
                            nc.vector.tensor_copy(
                                xT[dt][:, nt * P : (nt + 1) * P], ps[:]
                            )

                # ---- Phase C: v = x @ Wv (lhsT = xT), interleave ones cols ----
                with (
                    tc.tile_pool(name="wv", bufs=1) as wvp,
                    tc.tile_pool(name="psv", bufs=2, space="PSUM") as psv,
                ):
                    wv = []
                    for dt in range(T):
                        w_sb = wvp.tile([P, DIM], F32R, tag=f"wv{dt}")
                        nc.sync.dma_start(
                            w_sb[:], wqkv_ext[dt * P : (dt + 1) * P, 2 * DIM : 3 * DIM].bitcast(F32R)
                        )
                        wv.append(w_sb)
                    for nt in range(T):
                        ps = psv.tile([P, DIM], F32, tag="psv")
                        for dt in range(T):
                            lhsT = r(xT[dt][:, nt * P : (nt + 1) * P])
                            for c in (0, 512):
                                nc.tensor.matmul(
                                    ps[:, c : c + 512],
                                    lhsT,
                                    r(wv[dt][:, c : c + 512]),
                                    start=(dt == 0),
                                    stop=(dt == T - 1),
                                )
                        nc.gpsimd.memset(v[nt][:].bitcast(F32), 1.0)
                        nc.vector.tensor_copy(
                            v[nt][:, :, 0:DH],
                            ps[:].rearrange("p (h c) -> p h c", c=DH),
                        )

                # ---- Phases B+D: per half (4 head-pairs each) ----
                with (
                    tc.tile_pool(name="qk", bufs=1) as qkp,
                    tc.tile_pool(name="attn", bufs=1) as attnp,
                    tc.tile_pool(name="wqk", bufs=1) as wqkp,
                    tc.tile_pool(name="small", bufs=1) as small,
                    tc.tile_pool(name="psqk", bufs=2, space="PSUM") as psqk,
                    tc.tile_pool(name="pss", bufs=2, space="PSUM") as pss,
                    tc.tile_pool(name="psav", bufs=1, space="PSUM") as psav,
                ):
                    for half in range(2):
                        # B: qkT a-tiles for this half's 4 head-pairs.
                        # q features: cols half*512..+512 of w_qkv; k: 1024+half*512..
                        wq, wk = [], []
                        for dt in range(T):
                            wq_sb = wqkp.tile([P, 512], F32R, tag=f"wq{dt}")
                            nc.sync.dma_start(
                                wq_sb[:],
                                wqkv_ext[
                                    dt * P : (dt + 1) * P,
                                    half * 512 : half * 512 + 512,
                                ].bitcast(F32R),
                            )
                            wq.append(wq_sb)
                            wk_sb = wqkp.tile([P, 512], F32R, tag=f"wk{dt}")
                            nc.sync.dma_start(
                                wk_sb[:],
                                wqkv_ext[
                                    dt * P : (dt + 1) * P,
                                    DIM + half * 512 : DIM + half * 512 + 512,
                                ].bitcast(F32R),
                            )
                            wk.append(wk_sb)

                        qt, kt = [], []
                        for al in range(4):  # local a-tile (one head pair each)
                            q_sb = qkp.tile([P, N], F32R, tag=f"q{al}")
                            k_sb = qkp.tile([P, N], F32R, tag=f"k{al}")
                            for w_tiles, dst in ((wq, q_sb), (wk, k_sb)):
                                for c in (0, 512):
                                    ps = psqk.tile([P, 512], F32, tag="psqk")
                                    for dt in range(T):
                                        nc.tensor.matmul(
                                            ps[:],
                                            r(w_tiles[dt][:, al * P : (al + 1) * P]),
                                            r(xT[dt][:, c : c + 512]),
                                            start=(dt == 0),
                                            stop=(dt == T - 1),
                                        )
                                    nc.vector.tensor_copy(dst[:, c : c + 512], ps[:])
                            qt.append(q_sb)
                            kt.append(k_sb)

                        # D: attention for heads 2*hp, 2*hp+1 (hp = half*4+al)
                        for al in range(4):
                            hp = half * 4 + al
                            for sub in range(2):
                                h = 2 * hp + sub
                                ro = sub * DH  # partition offset of this head
                                q_h = qt[al][ro : ro + DH, :]
                                k_h = kt[al][ro : ro + DH, :]
                                attn_tiles = []
                                for jt in range(T):
                                    ps_s = pss.tile([P, N], F32, tag="pss")
                                    for c in (0, 512):
                                        nc.tensor.matmul(
                                            ps_s[:, c : c + 512],
                                            r(k_h[:, jt * P : (jt + 1) * P]),
                                            r(q_h[:, c : c + 512]),
                                            start=True,
                                            stop=True,
                                        )
                                    at_sb = attnp.tile([P, N], F32R, tag=f"at{jt}")
                                    nc.scalar.activation(
                                        at_sb[:], ps_s[:], Exp, scale=SCALE
                                    )
                                    attn_tiles.append(at_sb)
                                ps_av = psav.tile([DH + 1, N], F32, tag="psav")
                                for jt in range(T):
                                    lhsT = r(v[jt][:, h, :])  # [128, 65]
                                    for c in (0, 512):
                                        nc.tensor.matmul(
                                            ps_av[:, c : c + 512],
                                            lhsT,
                                            r(attn_tiles[jt][:, c : c + 512]),
                                            start=(jt == 0),
                                            stop=(jt == T - 1),
                                        )
                                recip = small.tile([1, N], F32, tag="recip")
                                nc.vector.reciprocal(recip[:], ps_av[DH : DH + 1, :])
                                recip_b = small.tile([DH, N], F32, tag="recipb")
                                nc.gpsimd.partition_broadcast(recip_b[:], recip[:])
                                nc.vector.tensor_tensor(
                                    mergedT[hp][ro : ro + DH, :],
                                    ps_av[0:DH, :],
                                    recip_b[:],
                                    mult,
                                )

            # ---- Phase E: out = mergedT.T @ Wout + b_out ----
            with (
                tc.tile_pool(name="wout", bufs=1) as woutp,
                tc.tile_pool(name="outp", bufs=3) as outp,
                tc.tile_pool(name="psf", bufs=2, space="PSUM") as psf,
            ):
                b_sb = outp.tile([1, DIM], F32, tag="b_sb")
                nc.sync.dma_start(b_sb[:], bout_ext[:])
                b_bcast = outp.tile([P, DIM], F32, tag="b_bcast")
                nc.gpsimd.partition_broadcast(b_bcast[:], b_sb[:])
                wout = []
                for at in range(T):
                    w_sb = woutp.tile([P, DIM], F32R, tag=f"wo{at}")
                    nc.sync.dma_start(w_sb[:], wout_ext[at * P : (at + 1) * P, :].bitcast(F32R))
                    wout.append(w_sb)
                for nt in range(T):
                    ps = psf.tile([P, DIM], F32, tag="psf")
                    for at in range(T):
                        lhsT = r(mergedT[at][:, nt * P : (nt + 1) * P])
                        for c in (0, 512):
                            nc.tensor.matmul(
                                ps[:, c : c + 512],
                                lhsT,
                                r(wout[at][:, c : c + 512]),
                                start=(at == 0),
                                stop=(at == T - 1),
                            )
                    o_sb = outp.tile([P, DIM], F32, tag="o_sb")
                    nc.vector.tensor_tensor(o_sb[:], ps[:], b_bcast[:], add)
                    nc.sync.dma_start(out_ext[nt * P : (nt + 1) * P, :], o_sb[:])

    nc.compile()
    return nc


def _get_nc():
    if "nc" not in _CACHE:
        _CACHE["nc"] = _build()
    return _CACHE["nc"]


def run(inputs, trace=False):
    from concourse.bass_utils import run_bass_kernel_spmd

    nc = _get_nc()
    x = np.ascontiguousarray(inputs["x"], dtype=np.float32)
    w_qkv = np.ascontiguousarray(inputs["w_qkv"], dtype=np.float32)
    w_out = np.ascontiguousarray(inputs["w_out"], dtype=np.float32)
    b_out = np.ascontiguousarray(inputs["b_out"], dtype=np.float32).reshape(1, DIM)
    in_maps = [
        {"x": x[i], "w_qkv": w_qkv, "w_out": w_out, "b_out": b_out} for i in range(B)
    ]
    res = run_bass_kernel_spmd(nc, in_maps, core_ids=list(range(B)), trace=trace)
    out = np.stack([res.results[i]["out"] for i in range(B)]).astype(np.float32)
    return out, res


def kernel(**inputs) -> np.ndarray:
    out, _ = run(inputs)
    return out


# revision 6
# speedup vs baseline: 4.0206x; 4.0206x over previous
"""Multi-head attention (B=8, N=1024, D=1024, H=16, Dh=64) on 8 TRN2 NeuronCores.

Sharding: pure data-parallel over batch — core i computes batch element i
end-to-end; weights are replicated. No collectives.

Per-core dataflow (all matmuls in float32r, full PE rate):
  A: xT = x.T                       (PE transpose via identity)
  C: v  = x @ Wv                    (lhsT = xT tiles)   [n, h, dv] +ones col
  B: qkT[a, n] = (x @ Wqk).T        (lhsT = Wqk tiles, rhs = xT)
  D: per head h: scoresT[j, i] = kT_h.T @ qT_h ; attnT = exp(scoresT/8) (ACT)
     avT[dv, i] = sum_j [v_h | 1] attnT  -> row 64 = softmax denominators
     mergedT[a, n] = avT * (1/denominator)   (DVE, gpsimd partition_broadcast)
  E: out[n, e] = mergedT.T @ Wout + b_out
"""

import sys

sys.path.insert(0, "/opt/trn_rl_repo")

import numpy as np

B, N, DIM = 8, 1024, 1024
HEADS, DH = 16, 64
P = 128
T = N // P  # 8 tiles per 1024 dim
SCALE = DH**-0.5

_CACHE = {}


def _emit_body(nc, tc, tile, mybir, x_ext, wqkv_ext, wout_ext, bout_ext, out_ext, sfx):
    """Emit one full attention pass reading x_ext, writing out_ext."""
    F32 = mybir.dt.float32
    F32R = mybir.dt.float32r
    Exp = mybir.ActivationFunctionType.Exp
    mult = mybir.AluOpType.mult
    add = mybir.AluOpType.add
    from concourse.masks import make_identity

    def r(ap):
        return ap.bitcast(F32R)

    with (
        tc.tile_pool(name=f"const{sfx}", bufs=1) as const,
        tc.tile_pool(name=f"merged{sfx}", bufs=1) as merged_pool,
    ):
        ident = const.tile([P, P], F32, tag="ident", name=f"ident{sfx}")
        make_identity(nc, ident)
        mergedT = [
            merged_pool.tile([P, N], F32R, tag=f"m{a}", name=f"m{a}{sfx}")
            for a in range(T)
        ]

        with (
            tc.tile_pool(name=f"xt{sfx}", bufs=1) as xt_pool,
            tc.tile_pool(name=f"vp{sfx}", bufs=1) as v_pool,
        ):
            xT = [
                xt_pool.tile([P, N], F32R, tag=f"xT{t}", name=f"xT{t}{sfx}")
                for t in range(T)
            ]
            v = [
                v_pool.tile([P, HEADS, DH + 1], F32R, tag=f"v{t}", name=f"v{t}{sfx}")
                for t in range(T)
            ]

            # ---- Phase A: transpose x -> xT ----
            with (
                tc.tile_pool(name=f"xin{sfx}", bufs=3) as xin,
                tc.tile_pool(name=f"pst{sfx}", bufs=4, space="PSUM") as pst,
            ):
                for nt in range(T):
                    x_sb = xin.tile([P, DIM], F32, tag="x_sb", name=f"x_sb{sfx}")
                    nc.sync.dma_start(x_sb[:], x_ext[nt * P : (nt + 1) * P, :])
                    for dt in range(T):
                        ps = pst.tile([P, P], F32, tag="pst", name=f"pst{sfx}")
                        nc.tensor.transpose(
                            ps[:], x_sb[:, dt * P : (dt + 1) * P], ident[:]
                        )
                        nc.vector.tensor_copy(xT[dt][:, nt * P : (nt + 1) * P], ps[:])

            # ---- Phase C: v = x @ Wv (lhsT = xT), interleaved ones cols ----
            with (
                tc.tile_pool(name=f"wv{sfx}", bufs=1) as wvp,
                tc.tile_pool(name=f"psv{sfx}", bufs=2, space="PSUM") as psv,
            ):
                wv = []
                for dt in range(T):
                    w_sb = wvp.tile([P, DIM], F32R, tag=f"wv{dt}", name=f"wv{dt}{sfx}")
                    nc.sync.dma_start(
                        w_sb[:],
                        wqkv_ext[dt * P : (dt + 1) * P, 2 * DIM : 3 * DIM].bitcast(
                            F32R
                        ),
                    )
                    wv.append(w_sb)
                for nt in range(T):
                    ps = psv.tile([P, DIM], F32, tag="psv", name=f"psv{sfx}")
                    for dt in range(T):
                        lhsT = r(xT[dt][:, nt * P : (nt + 1) * P])
                        for c in (0, 512):
                            nc.tensor.matmul(
                                ps[:, c : c + 512],
                                lhsT,
                                wv[dt][:, c : c + 512],
                                start=(dt == 0),
                                stop=(dt == T - 1),
                            )
                    nc.gpsimd.memset(v[nt][:].bitcast(F32), 1.0)
                    nc.vector.tensor_copy(
                        v[nt][:, :, 0:DH],
                        ps[:].rearrange("p (h c) -> p h c", c=DH),
                    )

            # ---- Phases B+D: per half (4 head-pairs each) ----
            with (
                tc.tile_pool(name=f"qk{sfx}", bufs=1) as qkp,
                tc.tile_pool(name=f"attn{sfx}", bufs=1) as attnp,
                tc.tile_pool(name=f"wqk{sfx}", bufs=1) as wqkp,
                tc.tile_pool(name=f"small{sfx}", bufs=1) as small,
                tc.tile_pool(name=f"psqk{sfx}", bufs=2, space="PSUM") as psqk,
                tc.tile_pool(name=f"pss{sfx}", bufs=2, space="PSUM") as pss,
                tc.tile_pool(name=f"psav{sfx}", bufs=1, space="PSUM") as psav,
            ):
                for half in range(2):
                    wq, wk = [], []
                    for dt in range(T):
                        wq_sb = wqkp.tile(
                            [P, 512], F32R, tag=f"wq{dt}", name=f"wq{dt}{sfx}"
                        )
                        nc.sync.dma_start(
                            wq_sb[:],
                            wqkv_ext[
                                dt * P : (dt + 1) * P,
                                half * 512 : half * 512 + 512,
                            ].bitcast(F32R),
                        )
                        wq.append(wq_sb)
                        wk_sb = wqkp.tile(
                            [P, 512], F32R, tag=f"wk{dt}", name=f"wk{dt}{sfx}"
                        )
                        nc.sync.dma_start(
                            wk_sb[:],
                            wqkv_ext[
                                dt * P : (dt + 1) * P,
                                DIM + half * 512 : DIM + half * 512 + 512,
                            ].bitcast(F32R),
                        )
                        wk.append(wk_sb)

                    qt, kt = [], []
                    for al in range(4):  # one head pair per a-tile
                        q_sb = qkp.tile([P, N], F32R, tag=f"q{al}", name=f"q{al}{sfx}")
                        k_sb = qkp.tile([P, N], F32R, tag=f"k{al}", name=f"k{al}{sfx}")
                        for w_tiles, dst in ((wq, q_sb), (wk, k_sb)):
                            for c in (0, 512):
                                ps = psqk.tile(
                                    [P, 512], F32, tag="psqk", name=f"psqk{sfx}"
                                )
                                for dt in range(T):
                                    nc.tensor.matmul(
                                        ps[:],
                                        r(w_tiles[dt][:, al * P : (al + 1) * P]),
                                        xT[dt][:, c : c + 512],
                                        start=(dt == 0),
                                        stop=(dt == T - 1),
                                    )
                                nc.vector.tensor_copy(dst[:, c : c + 512], ps[:])
                        qt.append(q_sb)
                        kt.append(k_sb)

                    for al in range(4):
                        hp = half * 4 + al
                        for sub in range(2):
                            h = 2 * hp + sub
                            ro = sub * DH
                            q_h = qt[al][ro : ro + DH, :]
                            k_h = kt[al][ro : ro + DH, :]
                            attn_tiles = []
                            for jt in range(T):
                                ps_s = pss.tile([P, N], F32, tag="pss", name=f"pss{sfx}")
                                for c in (0, 512):
                                    nc.tensor.matmul(
                                        ps_s[:, c : c + 512],
                                        r(k_h[:, jt * P : (jt + 1) * P]),
                                        q_h[:, c : c + 512],
                                        start=True,
                                        stop=True,
                                    )
                                at_sb = attnp.tile(
                                    [P, N], F32R, tag=f"at{jt}", name=f"at{jt}{sfx}"
                                )
                                nc.scalar.activation(at_sb[:], ps_s[:], Exp, scale=SCALE)
                                attn_tiles.append(at_sb)
                            ps_av = psav.tile(
                                [DH + 1, N], F32, tag="psav", name=f"psav{sfx}"
                            )
                            for jt in range(T):
                                lhsT = r(v[jt][:, h, :])  # [128, 65]
                                for c in (0, 512):
                                    nc.tensor.matmul(
                                        ps_av[:, c : c + 512],
                                        lhsT,
                                        attn_tiles[jt][:, c : c + 512],
                                        start=(jt == 0),
                                        stop=(jt == T - 1),
                                    )
                            recip = small.tile([1, N], F32, tag="recip", name=f"rc{sfx}")
                            nc.vector.reciprocal(recip[:], ps_av[DH : DH + 1, :])
                            recip_b = small.tile(
                                [DH, N], F32, tag="recipb", name=f"rb{sfx}"
                            )
                            nc.gpsimd.partition_broadcast(recip_b[:], recip[:])
                            nc.vector.tensor_tensor(
                                mergedT[hp][ro : ro + DH, :],
                                ps_av[0:DH, :],
                                recip_b[:],
                                mult,
                            )

        # ---- Phase E: out = mergedT.T @ Wout + b_out ----
        with (
            tc.tile_pool(name=f"wout{sfx}", bufs=1) as woutp,
            tc.tile_pool(name=f"outp{sfx}", bufs=3) as outp,
            tc.tile_pool(name=f"psf{sfx}", bufs=2, space="PSUM") as psf,
        ):
            b_sb = outp.tile([1, DIM], F32, tag="b_sb", name=f"b_sb{sfx}")
            nc.sync.dma_start(b_sb[:], bout_ext[:])
            b_bcast = outp.tile([P, DIM], F32, tag="b_bcast", name=f"b_bcast{sfx}")
            nc.gpsimd.partition_broadcast(b_bcast[:], b_sb[:])
            wout = []
            for at in range(T):
                w_sb = woutp.tile([P, DIM], F32R, tag=f"wo{at}", name=f"wo{at}{sfx}")
                nc.sync.dma_start(
                    w_sb[:], wout_ext[at * P : (at + 1) * P, :].bitcast(F32R)
                )
                wout.append(w_sb)
            for nt in range(T):
                ps = psf.tile([P, DIM], F32, tag="psf", name=f"psf{sfx}")
                for at in range(T):
                    lhsT = r(mergedT[at][:, nt * P : (nt + 1) * P])
                    for c in (0, 512):
                        nc.tensor.matmul(
                            ps[:, c : c + 512],
                            lhsT,
                            wout[at][:, c : c + 512],
                            start=(at == 0),
                            stop=(at == T - 1),
                        )
                o_sb = outp.tile([P, DIM], F32, tag="o_sb", name=f"o_sb{sfx}")
                nc.vector.tensor_tensor(o_sb[:], ps[:], b_bcast[:], add)
                nc.sync.dma_start(out_ext[nt * P : (nt + 1) * P, :], o_sb[:])


def _build(reps=1):
    import concourse.tile as tile
    from concourse import bacc, mybir

    F32 = mybir.dt.float32

    nc = bacc.Bacc("TRN2", target_bir_lowering=False, debug=False, num_devices=8)
    x_ext = nc.declare_dram_parameter("x", [N, DIM], F32, isOutput=False)
    wqkv_ext = nc.declare_dram_parameter("w_qkv", [DIM, 3 * DIM], F32, isOutput=False)
    wout_ext = nc.declare_dram_parameter("w_out", [DIM, DIM], F32, isOutput=False)
    bout_ext = nc.declare_dram_parameter("b_out", [1, DIM], F32, isOutput=False)
    out_ext = nc.declare_dram_parameter("out", [N, DIM], F32, isOutput=True)
    bounce = [
        nc.dram_tensor(f"bounce{k}", [N, DIM], F32) for k in range(max(0, reps - 1))
    ]

    with tile.TileContext(nc) as tc:
        for k in range(reps):
            src = x_ext if k == 0 else bounce[k - 1]
            dst = out_ext if k == reps - 1 else bounce[k]
            _emit_body(
                nc, tc, tile, mybir, src, wqkv_ext, wout_ext, bout_ext, dst, f"_{k}"
            )
    nc.compile()
    return nc


def _get_nc(reps=1):
    key = ("nc", reps)
    if key not in _CACHE:
        _CACHE[key] = _build(reps)
    return _CACHE[key]


def run(inputs, trace=False, reps=1):
    from concourse.bass_utils import run_bass_kernel_spmd

    nc = _get_nc(reps)
    x = np.ascontiguousarray(inputs["x"], dtype=np.float32)
    w_qkv = np.ascontiguousarray(inputs["w_qkv"], dtype=np.float32)
    w_out = np.ascontiguousarray(inputs["w_out"], dtype=np.float32)
    b_out = np.ascontiguousarray(inputs["b_out"], dtype=np.float32).reshape(1, DIM)
    in_maps = [
        {"x": x[i], "w_qkv": w_qkv, "w_out": w_out, "b_out": b_out} for i in range(B)
    ]
    res = run_bass_kernel_spmd(nc, in_maps, core_ids=list(range(B)), trace=trace)
    out = np.stack([res.results[i]["out"] for i in range(B)]).astype(np.float32)
    return out, res


def kernel(**inputs) -> np.ndarray:
    out, _ = run(inputs)
    return out


# revision 15
# speedup vs baseline: 4.6160x; 1.1481x over previous
"""Multi-head attention (B=8, N=1024, D=1024, H=16, Dh=64) on 8 TRN2 NeuronCores.

Sharding: pure data-parallel over batch — core i computes batch element i
end-to-end; weights are replicated. No collectives.

Per-core dataflow (PE compute dtype selectable: bf16 or float32r):
  A: xT = x.T                       (PE transpose via identity)
  C: v  = x @ Wv                    (lhsT = xT tiles)   [n, h, dv] +ones col
  B: qkT[a, n] = (x @ Wqk).T        (lhsT = Wqk tiles, rhs = xT)
  D: per head h: scoresT[j, i] = kT_h.T @ qT_h ; attnT = exp(scoresT/8) (ACT)
     avT[dv, i] = sum_j [v_h | 1] attnT  -> row 64 = softmax denominators
     mergedT[a, n] = avT * (1/denominator)   (DVE, gpsimd partition_broadcast)
  E: out[n, e] = mergedT.T @ Wout + b_out
"""

import sys

sys.path.insert(0, "/opt/trn_rl_repo")

import numpy as np

B, N, DIM = 8, 1024, 1024
HEADS, DH = 16, 64
P = 128
T = N // P  # 8 tiles per 1024 dim
SCALE = DH**-0.5

VARIANT = "bf16"  # "bf16" or "f32r"

_CACHE = {}


def _emit_body(
    nc, tc, tile, mybir, x_ext, wqkv_ext, wout_ext, bout_ext, out_ext, sfx, variant
):
    """Emit one full attention pass reading x_ext, writing out_ext."""
    F32 = mybir.dt.float32
    F32R = mybir.dt.float32r
    BF16 = mybir.dt.bfloat16
    Exp = mybir.ActivationFunctionType.Exp
    mult = mybir.AluOpType.mult
    add = mybir.AluOpType.add
    from concourse.masks import make_identity

    bf16 = variant == "bf16"
    CDT = BF16 if bf16 else F32R
    # moving-operand chunks per 1024-wide row: bf16 streams 1024, fp32r 512
    CHUNKS = [(0, 512), (512, 512)]

    def r(ap):
        # f32r tiles are declared F32R already; only DRAM f32 APs need a view
        return ap.bitcast(F32R)

    def load_w(pool, stage_pool, dram_ap, cols, tag):
        """Load a f32 weight slice into an SBUF tile of dtype CDT."""
        w_sb = pool.tile([P, cols], CDT, tag=tag, name=f"{tag}{sfx}")
        if bf16:
            st = stage_pool.tile([P, cols], F32, tag=f"st{cols}", name=f"st{sfx}")
            nc.sync.dma_start(st[:], dram_ap)
            nc.vector.tensor_copy(w_sb[:], st[:])
        else:
            nc.sync.dma_start(w_sb[:], dram_ap.bitcast(F32R))
        return w_sb

    with (
        tc.tile_pool(name=f"const{sfx}", bufs=1) as const,
        tc.tile_pool(name=f"merged{sfx}", bufs=1) as merged_pool,
        tc.tile_pool(name=f"stage{sfx}", bufs=3) as stage,
    ):
        ident = const.tile([P, P], F32, tag="ident", name=f"ident{sfx}")
        make_identity(nc, ident)
        # warm the ACT exp table set while PE does transposes
        warm = const.tile([1, 1], F32, tag="warm", name=f"warm{sfx}")
        nc.scalar.activation(warm[:], ident[0:1, 0:1], Exp)
        mergedT = [
            merged_pool.tile([P, N], CDT, tag=f"m{a}", name=f"m{a}{sfx}")
            for a in range(T)
        ]

        with (
            tc.tile_pool(name=f"xt{sfx}", bufs=1) as xt_pool,
            tc.tile_pool(name=f"vp{sfx}", bufs=1) as v_pool,
        ):
            xT = [
                xt_pool.tile([P, N], CDT, tag=f"xT{t}", name=f"xT{t}{sfx}")
                for t in range(T)
            ]
            v = [
                v_pool.tile([P, HEADS, DH + 1], CDT, tag=f"v{t}", name=f"v{t}{sfx}")
                for t in range(T)
            ]

            # ---- Phases A+C fused: per x row-tile, transpose then v-matmuls.
            # C(nt) only needs column block nt of every xT[dt], which the
            # transposes of x row nt just produced — interleaving fills PE
            # gaps between transposes with v-projection matmuls.
            with (
                tc.tile_pool(name=f"xin{sfx}", bufs=3) as xin,
                tc.tile_pool(name=f"wv{sfx}", bufs=1) as wvp,
                tc.tile_pool(name=f"pst{sfx}", bufs=4, space="PSUM") as pst,
                tc.tile_pool(name=f"psv{sfx}", bufs=4, space="PSUM") as psv,
            ):
                tid = ident
                if bf16:
                    identb = const.tile([P, P], BF16, tag="identb", name=f"idb{sfx}")
                    nc.vector.tensor_copy(identb[:], ident[:])
                    tid = identb
                wv = []
                for nt in range(T):
                    x_sb = xin.tile([P, DIM], F32, tag="x_sb", name=f"x_sb{sfx}")
                    nc.sync.dma_start(x_sb[:], x_ext[nt * P : (nt + 1) * P, :])
                    tin = x_sb
                    if bf16:
                        xb_sb = xin.tile([P, DIM], BF16, tag="xb_sb", name=f"xb{sfx}")
                        nc.vector.tensor_copy(xb_sb[:], x_sb[:])
                        tin = xb_sb
                    for dt in range(T):
                        ps = pst.tile([P, P], CDT if bf16 else F32, tag="pst",
                                      name=f"pst{sfx}")
                        nc.tensor.transpose(
                            ps[:], tin[:, dt * P : (dt + 1) * P], tid[:]
                        )
                        nc.vector.tensor_copy(xT[dt][:, nt * P : (nt + 1) * P], ps[:])
                    if nt == 0:
                        # emit weight loads after the first transposes so the
                        # scheduler starts PE on x-row-0 work, not on a wv wait
                        wv = [
                            load_w(
                                wvp,
                                stage,
                                wqkv_ext[dt * P : (dt + 1) * P, 2 * DIM : 3 * DIM],
                                DIM,
                                f"wv{dt}",
                            )
                            for dt in range(T)
                        ]
                    if bf16:
                        nc.gpsimd.memset(v[nt][:], 1.0)
                    else:
                        nc.gpsimd.memset(v[nt][:].bitcast(F32), 1.0)
                    for c, w in CHUNKS:
                        ps = psv.tile([P, 512], F32, tag="psv", name=f"psv{sfx}")
                        for dt in range(T):
                            nc.tensor.matmul(
                                ps[:],
                                xT[dt][:, nt * P : (nt + 1) * P],
                                wv[dt][:, c : c + w],
                                start=(dt == 0),
                                stop=(dt == T - 1),
                            )
                        nc.vector.tensor_copy(
                            v[nt][:, (c // DH) : (c // DH) + 8, 0:DH],
                            ps[:].rearrange("p (h c) -> p h c", c=DH),
                        )

            # ---- Phases B+D: per half (4 head-pairs each) ----
            with (
                tc.tile_pool(name=f"qk{sfx}", bufs=1) as qkp,
                tc.tile_pool(name=f"attn{sfx}", bufs=2 if bf16 else 1) as attnp,
                tc.tile_pool(name=f"wqk{sfx}", bufs=1) as wqkp,
                tc.tile_pool(name=f"small{sfx}", bufs=1) as small,
                tc.tile_pool(name=f"psqk{sfx}", bufs=2, space="PSUM") as psqk,
                tc.tile_pool(name=f"pss{sfx}", bufs=2, space="PSUM") as pss,
                tc.tile_pool(name=f"psav{sfx}", bufs=1, space="PSUM") as psav,
            ):
                def emit_av(hp, attn_tiles):
                    for sub in range(2):
                        h = 2 * hp + sub
                        ro = sub * DH
                        ps_av = psav.tile(
                            [DH + 1, N], F32, tag="psav", name=f"psav{sfx}"
                        )
                        for jt in range(T):
                            lhsT = v[jt][:, h, :]  # [128, 65]
                            for c, w in CHUNKS:
                                nc.tensor.matmul(
                                    ps_av[:, c : c + w],
                                    lhsT,
                                    attn_tiles[sub][jt][:, c : c + w],
                                    start=(jt == 0),
                                    stop=(jt == T - 1),
                                )
                        recip = small.tile([1, N], F32, tag="recip", name=f"rc{sfx}")
                        nc.vector.reciprocal(recip[:], ps_av[DH : DH + 1, :])
                        recip_b = small.tile([DH, N], F32, tag="recipb", name=f"rb{sfx}")
                        nc.gpsimd.partition_broadcast(recip_b[:], recip[:])
                        nc.vector.tensor_tensor(
                            mergedT[hp][ro : ro + DH, :],
                            ps_av[0:DH, :],
                            recip_b[:],
                            mult,
                        )

                slabs = {}
                pending = None  # (hp, attn_tiles) whose AV is deferred one pair
                for hp in range(8):
                    half, al = hp // 4, hp % 4
                    if al == 0:
                        wq, wk = [], []
                        for dt in range(T):
                            wq.append(
                                load_w(
                                    wqkp,
                                    stage,
                                    wqkv_ext[
                                        dt * P : (dt + 1) * P,
                                        half * 512 : half * 512 + 512,
                                    ],
                                    512,
                                    f"wq{dt}",
                                )
                            )
                            wk.append(
                                load_w(
                                    wqkp,
                                    stage,
                                    wqkv_ext[
                                        dt * P : (dt + 1) * P,
                                        DIM + half * 512 : DIM + half * 512 + 512,
                                    ],
                                    512,
                                    f"wk{dt}",
                                )
                            )
                        slabs[half] = (wq, wk)
                    wq, wk = slabs[half]

                    q_sb = qkp.tile([P, N], CDT, tag=f"q{al}", name=f"q{al}{sfx}")
                    k_sb = qkp.tile([P, N], CDT, tag=f"k{al}", name=f"k{al}{sfx}")
                    for w_tiles, dst in ((wq, q_sb), (wk, k_sb)):
                        for c, w in CHUNKS:
                            ps = psqk.tile([P, 512], F32, tag="psqk", name=f"psqk{sfx}")
                            for dt in range(T):
                                nc.tensor.matmul(
                                    ps[:],
                                    w_tiles[dt][:, al * P : (al + 1) * P],
                                    xT[dt][:, c : c + w],
                                    start=(dt == 0),
                                    stop=(dt == T - 1),
                                )
                            nc.vector.tensor_copy(dst[:, c : c + w], ps[:])

                    # scores for both heads of the pair issued adjacently:
                    # sub0 uses PE rows 0:64, sub1 rows 64:128 (distinct
                    # row-groups -> hardware runs them concurrently).
                    # f32r (tight SBUF) interleaves per sub instead.
                    attn_tiles = [[], []]
                    sub_order = (
                        [(jt, sub) for jt in range(T) for sub in range(2)]
                        if bf16
                        else [(jt, sub) for sub in range(2) for jt in range(T)]
                    )
                    for jt, sub in sub_order:
                        ro = sub * DH
                        ps_s = pss.tile([P, N], F32, tag="pss", name=f"pss{sfx}")
                        for c, w in CHUNKS:
                            nc.tensor.matmul(
                                ps_s[:, c : c + w],
                                k_sb[ro : ro + DH, jt * P : (jt + 1) * P],
                                q_sb[ro : ro + DH, c : c + w],
                                start=True,
                                stop=True,
                            )
                        at_sb = attnp.tile(
                            [P, N],
                            CDT,
                            tag=f"at{jt}_{sub}" if bf16 else f"at{jt}",
                            name=f"at{jt}_{sub}{sfx}",
                        )
                        nc.scalar.activation(at_sb[:], ps_s[:], Exp, scale=SCALE)
                        attn_tiles[sub].append(at_sb)

                    # defer this pair's AV so the NEXT pair's qk+scores stay
                    # ahead of it in PE order (keeps ACT fed at pair edges)
                    if pending is not None:
                        emit_av(*pending)
                    pending = (hp, attn_tiles)
                emit_av(*pending)

        # ---- Phase E: out = mergedT.T @ Wout + b_out ----
        with (
            tc.tile_pool(name=f"wout{sfx}", bufs=1) as woutp,
            tc.tile_pool(name=f"outp{sfx}", bufs=3) as outp,
            tc.tile_pool(name=f"psf{sfx}", bufs=2, space="PSUM") as psf,
        ):
            b_sb = outp.tile([1, DIM], F32, tag="b_sb", name=f"b_sb{sfx}")
            nc.sync.dma_start(b_sb[:], bout_ext[:])
            b_bcast = outp.tile([P, DIM], F32, tag="b_bcast", name=f"b_bcast{sfx}")
            nc.gpsimd.partition_broadcast(b_bcast[:], b_sb[:])
            wout = [
                load_w(
                    woutp, stage, wout_ext[at * P : (at + 1) * P, :], DIM, f"wo{at}"
                )
                for at in range(T)
            ]
            for nt in range(T):
                ps = psf.tile([P, DIM], F32, tag="psf", name=f"psf{sfx}")
                for at in range(T):
                    lhsT = mergedT[at][:, nt * P : (nt + 1) * P]
                    for c, w in CHUNKS:
                        nc.tensor.matmul(
                            ps[:, c : c + w],
                            lhsT,
                            wout[at][:, c : c + w],
                            start=(at == 0),
                            stop=(at == T - 1),
                        )
                o_sb = outp.tile([P, DIM], F32, tag="o_sb", name=f"o_sb{sfx}")
                nc.vector.tensor_tensor(o_sb[:], ps[:], b_bcast[:], add)
                nc.sync.dma_start(out_ext[nt * P : (nt + 1) * P, :], o_sb[:])


def _build(reps=1, variant=None):
    import concourse.tile as tile
    from concourse import bacc, mybir

    if variant is None:
        variant = VARIANT
    F32 = mybir.dt.float32

    nc = bacc.Bacc("TRN2", target_bir_lowering=False, debug=False, num_devices=8)
    x_ext = nc.declare_dram_parameter("x", [N, DIM], F32, isOutput=False)
    wqkv_ext = nc.declare_dram_parameter("w_qkv", [DIM, 3 * DIM], F32, isOutput=False)
    wout_ext = nc.declare_dram_parameter("w_out", [DIM, DIM], F32, isOutput=False)
    bout_ext = nc.declare_dram_parameter("b_out", [1, DIM], F32, isOutput=False)
    out_ext = nc.declare_dram_parameter("out", [N, DIM], F32, isOutput=True)
    bounce = [
        nc.dram_tensor(f"bounce{k}", [N, DIM], F32) for k in range(max(0, reps - 1))
    ]

    with tile.TileContext(nc) as tc:
        for k in range(reps):
            src = x_ext if k == 0 else bounce[k - 1]
            dst = out_ext if k == reps - 1 else bounce[k]
            _emit_body(
                nc, tc, tile, mybir, src, wqkv_ext, wout_ext, bout_ext, dst,
                f"_{k}", variant,
            )
    nc.compile()
    return nc


def _get_nc(reps=1, variant=None):
    key = ("nc", reps, variant or VARIANT)
    if key not in _CACHE:
        _CACHE[key] = _build(reps, variant)
    return _CACHE[key]


def run(inputs, trace=False, reps=1, variant=None):
    from concourse.bass_utils import run_bass_kernel_spmd

    nc = _get_nc(reps, variant)
    x = np.ascontiguousarray(inputs["x"], dtype=np.float32)
    w_qkv = np.ascontiguousarray(inputs["w_qkv"], dtype=np.float32)
    w_out = np.ascontiguousarray(inputs["w_out"], dtype=np.float32)
    b_out = np.ascontiguousarray(inputs["b_out"], dtype=np.float32).reshape(1, DIM)
    in_maps = [
        {"x": x[i], "w_qkv": w_qkv, "w_out": w_out, "b_out": b_out} for i in range(B)
    ]
    res = run_bass_kernel_spmd(nc, in_maps, core_ids=list(range(B)), trace=trace)
    out = np.stack([res.results[i]["out"] for i in range(B)]).astype(np.float32)
    return out, res


def kernel(**inputs) -> np.ndarray:
    out, _ = run(inputs)
    return out
